# revision 13
# baseline (speedup 1.0000x reference)
"""Trainium2 Bass kernel for StyleGAN2-style modulated conv2d (ModConv2D).

Reference computation (per sample b):
    w      = kernel * (style[b] + 1)                 # modulate [3,3,Cin,Cout]
    w      = w / sqrt(sum(w^2, (kh,kw,Cin)) + 1e-8)  # demodulate per Cout
    y[b]   = conv2d_same(x[b], w)

Sharding: data-parallel over batch - 16 samples across 8 NeuronCores,
2 samples per core; the base kernel is replicated.

Algorithm: 1D Winograd F(4,3) along dy. Per sample the conv becomes
6 Winograd planes x 3 dx taps x 2 cin-chunks x 2 chunks x 2 cout-chunks
= 144 matmuls of N=512 (vs 288 for the direct 9-tap form):
  U'_p[dx]  (lhsT, fp16): integer dy-combos of the modulated taps
            U0=w0, U1=w0+w1+w2, U2=w0-w1+w2, U3=w0+2w1+4w2,
            U4=w0-2w1+4w2, U5=w2
  V'_p      (rhs, fp16):  scaled B^T row combos of 6-row x windows
            (scales S=[1/4,-1/6,-1/6,1/24,1/24,1] folded into V)
  M_p = sum_dx U'_p[dx] @ V'_p[.,c+dx]  (psum f32, dx=+-1 via the
            column-split trick: rhs [128,8,63], strided psum out)
  y[4rg+i] = sum_p A^T[i,p] M_p * d    (ACT evicts M*d fp16, DVE A-combos)

x ingest (fp32->fp16 SWDGE cast, PE transpose for the startup blocks /
DMA-xbar for the rest) and the output path (strided o_sb assembly,
xbar out-transpose, SWDGE fp32 store) follow the direct-conv kernel.
"""

import numpy as np

B, H, W, CIN, COUT, KH, KW = 16, 64, 64, 256, 256, 3, 3
NCORES = 8
BPC = B // NCORES  # samples per core
HWPIX = H * W  # 4096
PAD0 = 64  # one zero guard row before the image
XLEN = PAD0 + HWPIX + 256  # 4416: room for the row-strided V views (rows up to 67)
NCHUNK = 2  # chunks of 8 row-groups (512 psum cols) each

_CACHE = {}
LAST_EXEC_NS = None
LAST_MEAN_EXEC_NS = None


def _build_nc():
    from contextlib import ExitStack

    import concourse.bacc as bacc
    import concourse.bass as bass
    import concourse.mybir as mybir
    import concourse.tile as tile
    from concourse.masks import make_identity

    f32 = mybir.dt.float32
    f16 = mybir.dt.float16
    AF = mybir.ActivationFunctionType
    OP = mybir.AluOpType

    nc = bacc.Bacc("TRN2", target_bir_lowering=False, debug=False)

    x_d = nc.dram_tensor("x", [BPC, H, W, CIN], f32, kind="ExternalInput")
    s_d = nc.dram_tensor("style", [BPC, CIN], f32, kind="ExternalInput")
    k_d = nc.dram_tensor("kernel", [KH, KW, CIN, COUT], f32, kind="ExternalInput")
    y_d = nc.dram_tensor("y", [BPC, H, W, COUT], f32, kind="ExternalOutput")

    XB = H * W * CIN
    KKW = CIN * COUT

    def x_blk_ap(b, t8):
        # [128 pix, 2 cc, 4 sblk, 128 ci]: cc-major so xtmp[:, cc] is 2D-mergeable
        off = b * XB + t8 * 4 * 128 * CIN
        return bass.AP(
            x_d, off, [[CIN, 128], [128, 2], [128 * CIN, 4], [1, 128]]
        )

    def y_blk_ap(b, t8):
        off = b * XB + t8 * 4 * 128 * COUT
        return bass.AP(y_d, off, [[COUT, 128], [128 * COUT, 4], [1, COUT]])

    def k_tap_ap(cc, t):
        return bass.AP(k_d, t * KKW + cc * 128 * COUT, [[COUT, 128], [1, COUT]])

    # kernel tap DMA order: dy=0 row, then dy=2 (planes 0/5 are plain taps and
    # unblock the first conv planes), then dy=1
    KTAP_ORDER = [0, 1, 2, 6, 7, 8, 3, 4, 5]
    # conv plane order: view-planes first (their weights are ready earliest)
    PLANE_ORDER = [0, 5, 1, 2, 3, 4]

    def r3(a):
        return a.rearrange("p (r w) -> p r w", w=64)

    with tile.TileContext(nc) as tc, ExitStack() as ctx:
        singles = ctx.enter_context(tc.tile_pool(name="singles", bufs=1))
        tmp_pool = ctx.enter_context(tc.tile_pool(name="tmp", bufs=1))
        wmod_pool = ctx.enter_context(tc.tile_pool(name="wmod", bufs=1))
        upool = ctx.enter_context(tc.tile_pool(name="upool", bufs=2))
        dpool = ctx.enter_context(tc.tile_pool(name="dpool", bufs=2))
        srow_pool = ctx.enter_context(tc.tile_pool(name="srow", bufs=2))
        xpool = ctx.enter_context(tc.tile_pool(name="xpool", bufs=1))
        xtpool = ctx.enter_context(tc.tile_pool(name="xt", bufs=2 * 8))
        vpool = ctx.enter_context(tc.tile_pool(name="vpool", bufs=2))
        vtmp_pool = ctx.enter_context(tc.tile_pool(name="vtmp", bufs=1))
        mpool = ctx.enter_context(tc.tile_pool(name="mpool", bufs=12))
        ytmp_pool = ctx.enter_context(tc.tile_pool(name="ytmp", bufs=1))
        ospool = ctx.enter_context(tc.tile_pool(name="osb", bufs=2))
        obpool = ctx.enter_context(tc.tile_pool(name="ob", bufs=8))
        pconv = ctx.enter_context(tc.tile_pool(name="pconv", bufs=5, space="PSUM"))
        pxt = ctx.enter_context(tc.tile_pool(name="pxt", bufs=2, space="PSUM"))
        psmall = ctx.enter_context(tc.tile_pool(name="psmall", bufs=1, space="PSUM"))

        # style rows + kernel tap loads, alternating HWDGE rings
        srows = []
        for b in range(BPC):
            srow = srow_pool.tile([1, CIN], f32, tag="srow")
            nc.scalar.dma_start(out=srow, in_=s_d.ap()[b : b + 1, :])
            srows.append(srow)
        kbase = singles.tile([128, 2, KH * KW, COUT], f32)
        for ti, t in enumerate(KTAP_ORDER):
            for cc in range(2):
                eng = nc.sync if (ti * 2 + cc) % 2 == 0 else nc.scalar
                eng.dma_start(out=kbase[:, cc, t], in_=k_tap_ap(cc, t))

        # x loads (fp32->fp16 SWDGE cast) issued upfront
        xts = [[None] * 8 for _ in range(BPC)]

        def load_xtmp(b, t8):
            xtmp = xtpool.tile([128, 2, 4, 128], f16, tag="xtmp", name=f"xtmp_{b}_{t8}")
            nc.gpsimd.dma_start(out=xtmp, in_=x_blk_ap(b, t8))
            xts[b][t8] = xtmp

        load_xtmp(0, 0)
        load_xtmp(0, 1)
        ident_b = singles.tile([128, 128], f16)
        make_identity(nc, ident_b)
        for b in range(BPC):
            for t8 in range(8):
                if xts[b][t8] is None:
                    load_xtmp(b, t8)

        ones1 = singles.tile([1, 1], f32)
        nc.vector.memset(ones1, 1.0)
        eps_sb = singles.tile([128, 1], f32)
        nc.vector.memset(eps_sb, 1e-8)

        # K2[cin, cout] = sum_t kernel^2 (for the demod factor)
        k2 = singles.tile([128, 2, COUT], f32)
        for cc in range(2):
            k2tmp = tmp_pool.tile([128, KH * KW, COUT], f32)
            nc.vector.tensor_mul(k2tmp, kbase[:, cc], kbase[:, cc])
            nc.vector.reduce_sum(
                out=k2[:, cc],
                in_=k2tmp.rearrange("p t c -> p c t"),
                axis=mybir.AxisListType.X,
            )

        # ---- per-sample modulation, U' planes, demod factors ----
        ups, dsbs = [], []
        for b in range(BPC):
            srow1 = srow_pool.tile([1, CIN], f32, tag="srow1")
            nc.vector.tensor_scalar_add(srow1, srows[b], 1.0)

            smod = dpool.tile([128, 2], f32)
            s2c = dpool.tile([128, 2], f32)
            for cc in range(2):
                pcol = psmall.tile([128, 1], f32, tag="psmall")
                nc.tensor.matmul(
                    pcol, srow1[:, cc * 128 : (cc + 1) * 128], ones1, start=True, stop=True
                )
                nc.vector.tensor_copy(out=smod[:, cc : cc + 1], in_=pcol)
            nc.vector.tensor_mul(s2c, smod, smod)

            # modulate: wmod16 fp16 feeds the U combos; dy0/dy2 taps also go
            # straight into their up slots (they ARE planes 0 and 5)
            wmod16 = wmod_pool.tile([128, 2, KH * KW, COUT], f16, tag="wmod16")
            up = upool.tile([128, 2, KW, 6, COUT], f16, tag="up")  # [cin, cc, dx, plane, cout]
            for t in KTAP_ORDER:
                dy, dx = t // 3, t % 3
                for cc in range(2):
                    nc.scalar.activation(
                        wmod16[:, cc, t], kbase[:, cc, t], AF.Copy,
                        scale=smod[:, cc : cc + 1],
                    )
                    if dy == 0:
                        nc.scalar.activation(
                            up[:, cc, dx, 0], kbase[:, cc, t], AF.Copy,
                            scale=smod[:, cc : cc + 1],
                        )
                    elif dy == 2:
                        nc.scalar.activation(
                            up[:, cc, dx, 5], kbase[:, cc, t], AF.Copy,
                            scale=smod[:, cc : cc + 1],
                        )
            # U' combos on DVE, whole dy-rows at a time ([128, 3dx, 256] fp16)
            for cc in range(2):
                w0 = wmod16[:, cc, 0:3]
                w1 = wmod16[:, cc, 3:6]
                w2 = wmod16[:, cc, 6:9]

                def upl(p):
                    return up[:, cc, :, p, :]

                t02 = tmp_pool.tile([128, 3, COUT], f16, tag="u_t02")
                nc.vector.tensor_add(t02, w0, w2)
                nc.vector.tensor_add(upl(1), t02, w1)
                nc.vector.tensor_sub(upl(2), t02, w1)
                ua = tmp_pool.tile([128, 3, COUT], f16, tag="u_a")
                nc.vector.scalar_tensor_tensor(
                    out=ua, in0=w1, scalar=2.0, in1=w0, op0=OP.mult, op1=OP.add
                )
                nc.vector.scalar_tensor_tensor(
                    out=upl(3), in0=w2, scalar=4.0, in1=ua, op0=OP.mult, op1=OP.add
                )
                ub = tmp_pool.tile([128, 3, COUT], f16, tag="u_b")
                nc.vector.scalar_tensor_tensor(
                    out=ub, in0=w1, scalar=-2.0, in1=w0, op0=OP.mult, op1=OP.add
                )
                nc.vector.scalar_tensor_tensor(
                    out=upl(4), in0=w2, scalar=4.0, in1=ub, op0=OP.mult, op1=OP.add
                )
            ups.append(up)

            # demod d[cout] = rsqrt(sum_cc s2c^T @ k2 + 1e-8)
            prow = psmall.tile([1, COUT], f32, tag="psmall")
            for cc in range(2):
                nc.tensor.matmul(
                    prow, s2c[:, cc : cc + 1], k2[:, cc], start=(cc == 0), stop=(cc == 1)
                )
            ssq_row = srow_pool.tile([1, COUT], f32, tag="ssq")
            nc.vector.tensor_copy(out=ssq_row, in_=prow)
            sqc = dpool.tile([128, 2], f32)
            for oc in range(2):
                pcol2 = psmall.tile([128, 1], f32, tag="psmall")
                nc.tensor.matmul(
                    pcol2, ssq_row[:, oc * 128 : (oc + 1) * 128], ones1, start=True, stop=True
                )
                nc.scalar.activation(sqc[:, oc : oc + 1], pcol2, AF.Sqrt, bias=eps_sb)
            d_sb = dpool.tile([128, 2], f32)
            nc.vector.reciprocal(d_sb, sqc)
            dsbs.append(d_sb)

        for b in range(BPC):
            up = ups[b]
            d_sb = dsbs[b]
            # x channel-major flat: [128 cin, cc, XLEN] fp16; guard rows zero.
            # Single buffer: sample b+1's transposes depend on b's V reads.
            xflat = xpool.tile([128, 2, XLEN], f16, tag="xflat")
            if b == 0:
                nc.vector.memset(xflat[:, :, 0:PAD0], 0.0)
                nc.vector.memset(xflat[:, :, PAD0 + HWPIX : XLEN], 0.0)

            def transpose_block_pe(t8):
                xtmp = xts[b][t8]
                for cc in range(2):
                    pxt_t = pxt.tile([128, 4, 128], f16, tag="pxt")
                    for s4 in range(4):
                        nc.tensor.transpose(
                            pxt_t[:, s4, :], xtmp[:, cc, s4, :], ident_b
                        )
                    nc.vector.tensor_copy(
                        out=xflat[:, cc, PAD0 + 512 * t8 : PAD0 + 512 * (t8 + 1)],
                        in_=pxt_t,
                    )

            def transpose_block_xbar(t8):
                xtmp = xts[b][t8]
                for cc in range(2):
                    eng = nc.sync if cc == 0 else nc.scalar
                    eng.dma_start_transpose(
                        out=xflat[:, cc, PAD0 + 512 * t8 : PAD0 + 512 * (t8 + 1)],
                        in_=xtmp[:, cc],
                    )

            def build_v_chunk(c):
                # V' planes for row-groups 8c..8c+7: 18 fp16 DVE passes per cc
                # over [128, 8, 64] row-strided views of xflat (guard rows
                # supply the SAME padding); walrus caps DVE APs at 3 dims
                vt = vpool.tile([128, 6, 2, 512], f16, tag="vt", name=f"vt_{b}_{c}")
                for cc in range(2):
                    xfc = xflat[:, cc]

                    def drow(k):
                        # rows 4rg + k - 1 for rg = 8c..8c+7 -> [128, 8(stride 256), 64]
                        base = PAD0 + (32 * c + k - 1) * 64
                        return xfc[:, base : base + 2048].rearrange(
                            "p (r f w) -> p r f w", f=4, w=64
                        )[:, :, 0, :]

                    def vplane(p):
                        return r3(vt[:, p, cc])

                    d0, d1, d2, d3, d4, d5 = (drow(k) for k in range(6))
                    tt = vtmp_pool.tile([128, 8, 64], f16, tag="v_t")
                    uu = vtmp_pool.tile([128, 8, 64], f16, tag="v_u")
                    vv = vtmp_pool.tile([128, 8, 64], f16, tag="v_v")
                    # p0 = d0 - 1.25 d2 + 0.25 d4
                    nc.vector.scalar_tensor_tensor(
                        out=tt, in0=d2, scalar=-5.0, in1=d4, op0=OP.mult, op1=OP.add
                    )
                    nc.vector.scalar_tensor_tensor(
                        out=vplane(0), in0=tt, scalar=0.25, in1=d0, op0=OP.mult, op1=OP.add
                    )
                    # p1 = (4(d1+d2) - (d3+d4))/6
                    nc.vector.tensor_add(tt, d1, d2)
                    nc.vector.tensor_add(uu, d3, d4)
                    nc.vector.scalar_tensor_tensor(
                        out=vv, in0=tt, scalar=4.0, in1=uu, op0=OP.mult, op1=OP.subtract
                    )
                    nc.vector.tensor_scalar_mul(vplane(1), vv, 1.0 / 6.0)
                    # p2 = (4(d2-d1) + (d3-d4))/6
                    nc.vector.tensor_sub(tt, d2, d1)
                    nc.vector.tensor_sub(uu, d3, d4)
                    nc.vector.scalar_tensor_tensor(
                        out=vv, in0=tt, scalar=4.0, in1=uu, op0=OP.mult, op1=OP.add
                    )
                    nc.vector.tensor_scalar_mul(vplane(2), vv, 1.0 / 6.0)
                    # p3 = (2(d3-d1) + (d4-d2))/24 ; p4 = (-2(d3-d1) + (d4-d2))/24
                    nc.vector.tensor_sub(tt, d3, d1)
                    nc.vector.tensor_sub(uu, d4, d2)
                    nc.vector.scalar_tensor_tensor(
                        out=vv, in0=tt, scalar=2.0, in1=uu, op0=OP.mult, op1=OP.add
                    )
                    nc.vector.tensor_scalar_mul(vplane(3), vv, 1.0 / 24.0)
                    nc.vector.scalar_tensor_tensor(
                        out=vv, in0=tt, scalar=-2.0, in1=uu, op0=OP.mult, op1=OP.add
                    )
                    nc.vector.tensor_scalar_mul(vplane(4), vv, 1.0 / 24.0)
                    # p5 = 4 d1 - 5 d3 + d5
                    nc.vector.scalar_tensor_tensor(
                        out=tt, in0=d3, scalar=-5.0, in1=d5, op0=OP.mult, op1=OP.add
                    )
                    nc.vector.scalar_tensor_tensor(
                        out=vplane(5), in0=d1, scalar=4.0, in1=tt, op0=OP.mult, op1=OP.add
                    )
                return vt

            # output ob tiles, shared between the oc halves
            obs = {}
            for t8 in range(8):
                obs[t8] = obpool.tile([128, 4, COUT], f16, tag="ob", name=f"ob_{b}_{t8}")

            def evict_chunk(c, oc, mp):
                # mp: 6 sbuf fp16 tiles [128, 512] (already scaled by d)
                # y0 = m0+s+u, y1 = t+2v, y2 = s+4u, y3 = t+8v+m5
                # with s = m1+m2, t = m1-m2, u = m3+m4, v = m3-m4
                s_ = ytmp_pool.tile([128, 512], f16, tag="y_s")
                t_ = ytmp_pool.tile([128, 512], f16, tag="y_t")
                u_ = ytmp_pool.tile([128, 512], f16, tag="y_u")
                v_ = ytmp_pool.tile([128, 512], f16, tag="y_v")
                nc.vector.tensor_add(s_, mp[1], mp[2])
                nc.vector.tensor_sub(t_, mp[1], mp[2])
                nc.vector.tensor_add(u_, mp[3], mp[4])
                nc.vector.tensor_sub(v_, mp[3], mp[4])
                o_sb = ospool.tile([128, 4 * 512], f16, tag="osb")
                o_v = o_sb.rearrange("p (r i w) -> p r i w", i=4, w=64)
                t1 = ytmp_pool.tile([128, 512], f16, tag="y_t1")
                nc.vector.tensor_add(t1, mp[0], s_)
                nc.vector.tensor_add(o_v[:, :, 0], r3(t1), r3(u_))
                nc.vector.scalar_tensor_tensor(
                    out=o_v[:, :, 1], in0=r3(v_), scalar=2.0, in1=r3(t_),
                    op0=OP.mult, op1=OP.add,
                )
                nc.vector.scalar_tensor_tensor(
                    out=o_v[:, :, 2], in0=r3(u_), scalar=4.0, in1=r3(s_),
                    op0=OP.mult, op1=OP.add,
                )
                t2 = ytmp_pool.tile([128, 512], f16, tag="y_t2")
                nc.vector.scalar_tensor_tensor(
                    out=t2, in0=v_, scalar=8.0, in1=t_, op0=OP.mult, op1=OP.add,
                )
                nc.vector.tensor_add(o_v[:, :, 3], r3(t2), r3(mp[5]))
                # out-transpose + store, 512 px at a time
                last = b == BPC - 1 and c == NCHUNK - 1 and oc == 1
                for q in range(4):
                    t8 = c * 4 + q
                    osq = o_sb[:, q * 512 : (q + 1) * 512]
                    ob = obs[t8]
                    if last and q == 3:
                        # final tile: PE transpose (ingest psum pool idle now)
                        pot_t = pxt.tile([128, 4, 128], f16, tag="pxt")
                        for s4 in range(4):
                            nc.tensor.transpose(
                                pot_t[:, s4, :], osq[:, s4 * 128 : (s4 + 1) * 128], ident_b
                            )
                        nc.vector.tensor_copy(
                            out=ob[:, :, oc * 128 : (oc + 1) * 128], in_=pot_t
                        )
                        nc.gpsimd.dma_start(out=y_blk_ap(b, t8), in_=ob)
                    else:
                        eng = nc.sync if oc == 0 else nc.scalar
                        eng.dma_start_transpose(
                            out=ob[:, :, oc * 128 : (oc + 1) * 128], in_=osq
                        )
                        if oc == 1:
                            nc.gpsimd.dma_start(out=y_blk_ap(b, t8), in_=ob)

            def conv_chunk(c, oc, vt):
                mp = [None] * 6
                for p in PLANE_ORDER:
                    ps = pconv.tile([128, 512], f32, tag="pconv")
                    ps_r = r3(ps)
                    i = 0
                    for dx in [0, -1, 1]:  # dx=0 first: start=True covers all 512
                        for cc in range(2):
                            lhsT = up[:, cc, dx + 1, p, oc * 128 : (oc + 1) * 128]
                            vpl = r3(vt[:, p, cc])
                            if dx == 0:
                                rhs = vt[:, p, cc]
                                out_ap = ps
                            elif dx == -1:
                                rhs = vpl[:, :, 0:63]
                                out_ap = ps_r[:, :, 1:64]
                            else:
                                rhs = vpl[:, :, 1:64]
                                out_ap = ps_r[:, :, 0:63]
                            nc.tensor.matmul(
                                out_ap, lhsT, rhs, start=(i == 0), stop=(i == 5)
                            )
                            i += 1
                    msb = mpool.tile([128, 512], f16, tag="msb", name=f"m_{p}")
                    nc.scalar.activation(msb, ps, AF.Copy, scale=d_sb[:, oc : oc + 1])
                    mp[p] = msb
                evict_chunk(c, oc, mp)

            # ingest via PE transpose (the baseline-proven path; the xbar
            # variant produced wrong results on HW)
            for t8 in range(5):
                transpose_block_pe(t8)
            vt0 = build_v_chunk(0)
            for t8 in range(5, 8):
                transpose_block_pe(t8)
            conv_chunk(0, 0, vt0)
            vt1 = build_v_chunk(1)
            conv_chunk(0, 1, vt0)
            conv_chunk(1, 0, vt1)
            conv_chunk(1, 1, vt1)

    nc.compile()
    return nc


def _get_nc():
    if "nc" not in _CACHE:
        _CACHE["nc"] = _build_nc()
    return _CACHE["nc"]


def kernel(x, style, kernel, _trace=False):
    global LAST_EXEC_NS, LAST_MEAN_EXEC_NS
    from concourse.bass_utils import run_bass_kernel_spmd

    x = np.ascontiguousarray(x, dtype=np.float32)
    style = np.ascontiguousarray(style, dtype=np.float32)
    kern = np.ascontiguousarray(kernel, dtype=np.float32)

    nc = _get_nc()
    in_maps = [
        {
            "x": x[i * BPC : (i + 1) * BPC],
            "style": style[i * BPC : (i + 1) * BPC],
            "kernel": kern,
        }
        for i in range(NCORES)
    ]
    res = run_bass_kernel_spmd(nc, in_maps, core_ids=list(range(NCORES)), trace=_trace)
    LAST_EXEC_NS = res.exec_time_ns
    LAST_MEAN_EXEC_NS = res.mean_exec_time_ns
    return np.concatenate([res.results[i]["y"] for i in range(NCORES)], axis=0)


# revision 14
# speedup vs baseline: 1.2386x; 1.2386x over previous
"""Trainium2 Bass kernel for StyleGAN2-style modulated conv2d (ModConv2D).

Reference computation (per sample b):
    w      = kernel * (style[b] + 1)                 # modulate [3,3,Cin,Cout]
    w      = w / sqrt(sum(w^2, (kh,kw,Cin)) + 1e-8)  # demodulate per Cout
    y[b]   = conv2d_same(x[b], w)

Sharding: data-parallel over batch - 16 samples across 8 NeuronCores,
2 samples per core; the base kernel is replicated.

Algorithm: 1D Winograd F(2,3) along dy. Per sample the conv becomes
4 Winograd planes x 3 dx taps x 2 cin-chunks x 4 chunks x 2 cout-chunks
= 192 matmuls of N=512 (vs 288 for the direct 9-tap form), with cheap
transforms (plain adds/subs, DVE-friendly):
  U planes (lhsT, fp16): U0=w0, U1=(w0+w1+w2)/2, U2=(w0-w1+w2)/2, U3=w2
  V planes (rhs,  fp16): per row-pair rg (rows 2rg-1..2rg+2 of x):
      V0=d0-d2, V1=d1+d2, V2=d2-d1, V3=d1-d3
  M_p = sum_dx U_p[dx] @ V_p[.,c+dx]   (psum f32; dx=+-1 via the
      column-split trick: rhs [128,8,63], strided psum out)
  y[2rg+0] = (M0+M1+M2) * d ; y[2rg+1] = (M1-M2-M3) * d
      (ACT evicts M*d to fp16, DVE combines into the strided o_sb)

x ingest (fp32->fp16 SWDGE cast + PE transpose) and the output path
(xbar out-transpose, SWDGE fp32 store) follow the direct-conv kernel.
"""

import numpy as np

B, H, W, CIN, COUT, KH, KW = 16, 64, 64, 256, 256, 3, 3
NCORES = 8
BPC = B // NCORES  # samples per core
HWPIX = H * W  # 4096
PAD0 = 64  # one zero guard row before the image
XLEN = PAD0 + HWPIX + 256  # room for the row-strided V views
NCHUNK = 4  # chunks of 8 row-pairs (512 psum cols, 16 output rows) each

_CACHE = {}
LAST_EXEC_NS = None
LAST_MEAN_EXEC_NS = None


def _build_nc():
    from contextlib import ExitStack

    import concourse.bacc as bacc
    import concourse.bass as bass
    import concourse.mybir as mybir
    import concourse.tile as tile
    from concourse.masks import make_identity

    f32 = mybir.dt.float32
    f16 = mybir.dt.float16
    AF = mybir.ActivationFunctionType
    OP = mybir.AluOpType

    nc = bacc.Bacc("TRN2", target_bir_lowering=False, debug=False)

    x_d = nc.dram_tensor("x", [BPC, H, W, CIN], f32, kind="ExternalInput")
    s_d = nc.dram_tensor("style", [BPC, CIN], f32, kind="ExternalInput")
    k_d = nc.dram_tensor("kernel", [KH, KW, CIN, COUT], f32, kind="ExternalInput")
    y_d = nc.dram_tensor("y", [BPC, H, W, COUT], f32, kind="ExternalOutput")

    XB = H * W * CIN
    KKW = CIN * COUT

    def x_blk_ap(b, t8):
        # [128 pix, 2 cc, 4 sblk, 128 ci]: cc-major so xtmp[:, cc] is 2D-mergeable
        off = b * XB + t8 * 4 * 128 * CIN
        return bass.AP(
            x_d, off, [[CIN, 128], [128, 2], [128 * CIN, 4], [1, 128]]
        )

    def y_blk_ap(b, t8):
        off = b * XB + t8 * 4 * 128 * COUT
        return bass.AP(y_d, off, [[COUT, 128], [128 * COUT, 4], [1, COUT]])

    def k_tap_ap(cc, t):
        return bass.AP(k_d, t * KKW + cc * 128 * COUT, [[COUT, 128], [1, COUT]])

    # kernel tap DMA order: dy=0 row, then dy=2 (planes 0/3 are plain taps and
    # unblock the first conv planes), then dy=1
    KTAP_ORDER = [0, 1, 2, 6, 7, 8, 3, 4, 5]
    # conv plane order: view-planes first (their weights are ready earliest)
    PLANE_ORDER = [0, 3, 1, 2]

    def r3(a):
        return a.rearrange("p (r w) -> p r w", w=64)

    with tile.TileContext(nc) as tc, ExitStack() as ctx:
        singles = ctx.enter_context(tc.tile_pool(name="singles", bufs=1))
        tmp_pool = ctx.enter_context(tc.tile_pool(name="tmp", bufs=1))
        wmod_pool = ctx.enter_context(tc.tile_pool(name="wmod", bufs=1))
        upool = ctx.enter_context(tc.tile_pool(name="upool", bufs=2))
        dpool = ctx.enter_context(tc.tile_pool(name="dpool", bufs=2))
        srow_pool = ctx.enter_context(tc.tile_pool(name="srow", bufs=2))
        xpool = ctx.enter_context(tc.tile_pool(name="xpool", bufs=1))
        xtpool = ctx.enter_context(tc.tile_pool(name="xt", bufs=2 * 8))
        vpool = ctx.enter_context(tc.tile_pool(name="vpool", bufs=2))
        mpool = ctx.enter_context(tc.tile_pool(name="mpool", bufs=8))
        ytmp_pool = ctx.enter_context(tc.tile_pool(name="ytmp", bufs=1))
        ospool = ctx.enter_context(tc.tile_pool(name="osb", bufs=2))
        obpool = ctx.enter_context(tc.tile_pool(name="ob", bufs=8))
        pconv = ctx.enter_context(tc.tile_pool(name="pconv", bufs=5, space="PSUM"))
        pxt = ctx.enter_context(tc.tile_pool(name="pxt", bufs=2, space="PSUM"))
        psmall = ctx.enter_context(tc.tile_pool(name="psmall", bufs=1, space="PSUM"))

        # style rows + kernel tap loads, alternating HWDGE rings
        srows = []
        for b in range(BPC):
            srow = srow_pool.tile([1, CIN], f32, tag="srow")
            nc.scalar.dma_start(out=srow, in_=s_d.ap()[b : b + 1, :])
            srows.append(srow)
        kbase = singles.tile([128, 2, KH * KW, COUT], f32)
        for ti, t in enumerate(KTAP_ORDER):
            for cc in range(2):
                eng = nc.sync if (ti * 2 + cc) % 2 == 0 else nc.scalar
                eng.dma_start(out=kbase[:, cc, t], in_=k_tap_ap(cc, t))

        # x loads (fp32->fp16 SWDGE cast) issued upfront
        xts = [[None] * 8 for _ in range(BPC)]

        def load_xtmp(b, t8):
            xtmp = xtpool.tile([128, 2, 4, 128], f16, tag="xtmp", name=f"xtmp_{b}_{t8}")
            nc.gpsimd.dma_start(out=xtmp, in_=x_blk_ap(b, t8))
            xts[b][t8] = xtmp

        load_xtmp(0, 0)
        load_xtmp(0, 1)
        ident_b = singles.tile([128, 128], f16)
        make_identity(nc, ident_b)
        for b in range(BPC):
            for t8 in range(8):
                if xts[b][t8] is None:
                    load_xtmp(b, t8)

        ones1 = singles.tile([1, 1], f32)
        nc.vector.memset(ones1, 1.0)
        eps_sb = singles.tile([128, 1], f32)
        nc.vector.memset(eps_sb, 1e-8)

        # K2[cin, cout] = sum_t kernel^2 (for the demod factor)
        k2 = singles.tile([128, 2, COUT], f32)
        for cc in range(2):
            k2tmp = tmp_pool.tile([128, KH * KW, COUT], f32)
            nc.vector.tensor_mul(k2tmp, kbase[:, cc], kbase[:, cc])
            nc.vector.reduce_sum(
                out=k2[:, cc],
                in_=k2tmp.rearrange("p t c -> p c t"),
                axis=mybir.AxisListType.X,
            )

        ups, dsbs = [None] * BPC, [None] * BPC

        def setup_sample(b):
            # modulation, U planes, demod factor for sample b
            srow1 = srow_pool.tile([1, CIN], f32, tag="srow1")
            nc.vector.tensor_scalar_add(srow1, srows[b], 1.0)

            smod = dpool.tile([128, 2], f32)
            s2c = dpool.tile([128, 2], f32)
            for cc in range(2):
                pcol = psmall.tile([128, 1], f32, tag="psmall")
                nc.tensor.matmul(
                    pcol, srow1[:, cc * 128 : (cc + 1) * 128], ones1, start=True, stop=True
                )
                nc.vector.tensor_copy(out=smod[:, cc : cc + 1], in_=pcol)
            nc.vector.tensor_mul(s2c, smod, smod)

            # modulate all taps to fp16; dy0/dy2 taps also ARE planes 0/3
            wmod16 = wmod_pool.tile([128, 2, KH * KW, COUT], f16, tag="wmod16")
            up = upool.tile([128, 2, KW, 4, COUT], f16, tag="up")  # [cin, cc, dx, plane, cout]
            for t in KTAP_ORDER:
                dy, dx = t // 3, t % 3
                for cc in range(2):
                    nc.scalar.activation(
                        wmod16[:, cc, t], kbase[:, cc, t], AF.Copy,
                        scale=smod[:, cc : cc + 1],
                    )
                    if dy == 0:
                        nc.scalar.activation(
                            up[:, cc, dx, 0], kbase[:, cc, t], AF.Copy,
                            scale=smod[:, cc : cc + 1],
                        )
                    elif dy == 2:
                        nc.scalar.activation(
                            up[:, cc, dx, 3], kbase[:, cc, t], AF.Copy,
                            scale=smod[:, cc : cc + 1],
                        )
            # U1/U2 combos on DVE, whole dy-rows at a time ([128, 3dx, 256] fp16)
            for cc in range(2):
                w0 = wmod16[:, cc, 0:3]
                w1 = wmod16[:, cc, 3:6]
                w2 = wmod16[:, cc, 6:9]
                w1h = tmp_pool.tile([128, 3, COUT], f16, tag="u_w1h")
                nc.vector.tensor_scalar_mul(w1h, w1, 0.5)
                t02 = tmp_pool.tile([128, 3, COUT], f16, tag="u_t02")
                nc.vector.tensor_add(t02, w0, w2)
                nc.vector.scalar_tensor_tensor(
                    out=up[:, cc, :, 1, :], in0=t02, scalar=0.5, in1=w1h,
                    op0=OP.mult, op1=OP.add,
                )
                nc.vector.scalar_tensor_tensor(
                    out=up[:, cc, :, 2, :], in0=t02, scalar=0.5, in1=w1h,
                    op0=OP.mult, op1=OP.subtract,
                )
            ups[b] = up

            # demod d[cout] = rsqrt(sum_cc s2c^T @ k2 + 1e-8)
            prow = psmall.tile([1, COUT], f32, tag="psmall")
            for cc in range(2):
                nc.tensor.matmul(
                    prow, s2c[:, cc : cc + 1], k2[:, cc], start=(cc == 0), stop=(cc == 1)
                )
            ssq_row = srow_pool.tile([1, COUT], f32, tag="ssq")
            nc.vector.tensor_copy(out=ssq_row, in_=prow)
            sqc = dpool.tile([128, 2], f32)
            for oc in range(2):
                pcol2 = psmall.tile([128, 1], f32, tag="psmall")
                nc.tensor.matmul(
                    pcol2, ssq_row[:, oc * 128 : (oc + 1) * 128], ones1, start=True, stop=True
                )
                nc.scalar.activation(sqc[:, oc : oc + 1], pcol2, AF.Sqrt, bias=eps_sb)
            d_sb = dpool.tile([128, 2], f32)
            nc.vector.reciprocal(d_sb, sqc)
            dsbs[b] = d_sb

        setup_sample(0)

        for b in range(BPC):
            up = ups[b]
            d_sb = dsbs[b]
            # x channel-major flat: [128 cin, cc, XLEN] fp16; guard rows zero.
            # Single buffer: sample b+1's transposes depend on b's V reads.
            xflat = xpool.tile([128, 2, XLEN], f16, tag="xflat")
            if b == 0:
                nc.vector.memset(xflat[:, :, 0:PAD0], 0.0)
                nc.vector.memset(xflat[:, :, PAD0 + HWPIX : XLEN], 0.0)

            def transpose_block_pe(t8):
                xtmp = xts[b][t8]
                for cc in range(2):
                    pxt_t = pxt.tile([128, 4, 128], f16, tag="pxt")
                    for s4 in range(4):
                        nc.tensor.transpose(
                            pxt_t[:, s4, :], xtmp[:, cc, s4, :], ident_b
                        )
                    nc.vector.tensor_copy(
                        out=xflat[:, cc, PAD0 + 512 * t8 : PAD0 + 512 * (t8 + 1)],
                        in_=pxt_t,
                    )

            def build_v_chunk(c):
                # V planes for row-pairs 8c..8c+7 (output rows 16c..16c+15):
                # 4 plain tensor ops per cc over [128, 8, 64] row-strided views
                # of xflat (guard rows supply the SAME padding)
                vt = vpool.tile([128, 4, 2, 512], f16, tag="vt", name=f"vt_{b}_{c}")
                for cc in range(2):
                    xfc = xflat[:, cc]

                    def drow(k):
                        # rows 2rg + k - 1 for rg = 8c..8c+7 -> [128, 8(stride 128), 64]
                        base = PAD0 + (16 * c + k - 1) * 64
                        return xfc[:, base : base + 1024].rearrange(
                            "p (r f w) -> p r f w", f=2, w=64
                        )[:, :, 0, :]

                    def vplane(p):
                        return r3(vt[:, p, cc])

                    d0, d1, d2, d3 = (drow(k) for k in range(4))
                    nc.vector.tensor_sub(vplane(0), d0, d2)
                    nc.vector.tensor_add(vplane(1), d1, d2)
                    nc.vector.tensor_sub(vplane(2), d2, d1)
                    nc.vector.tensor_sub(vplane(3), d1, d3)
                return vt

            # output ob tiles, shared between the oc halves
            obs = {}
            for t8 in range(8):
                obs[t8] = obpool.tile([128, 4, COUT], f16, tag="ob", name=f"ob_{b}_{t8}")

            def evict_chunk(c, oc, mp):
                # mp: 4 sbuf fp16 tiles [128, 512] (already scaled by d)
                # y(2rg+0) = m0+m1+m2 ; y(2rg+1) = m1-m2-m3
                s_ = ytmp_pool.tile([128, 512], f16, tag="y_s")
                t_ = ytmp_pool.tile([128, 512], f16, tag="y_t")
                nc.vector.tensor_add(s_, mp[1], mp[2])
                nc.vector.tensor_sub(t_, mp[1], mp[2])
                o_sb = ospool.tile([128, 2 * 512], f16, tag="osb")
                o_v = o_sb.rearrange("p (r i w) -> p r i w", i=2, w=64)
                nc.vector.tensor_add(o_v[:, :, 0], r3(mp[0]), r3(s_))
                nc.vector.tensor_sub(o_v[:, :, 1], r3(t_), r3(mp[3]))
                # out-transpose + store, 512 px at a time
                last = b == BPC - 1 and c == NCHUNK - 1 and oc == 1
                for q in range(2):
                    t8 = c * 2 + q
                    osq = o_sb[:, q * 512 : (q + 1) * 512]
                    ob = obs[t8]
                    if last and q == 1:
                        # final tile: PE transpose (ingest psum pool idle now)
                        pot_t = pxt.tile([128, 4, 128], f16, tag="pxt")
                        for s4 in range(4):
                            nc.tensor.transpose(
                                pot_t[:, s4, :], osq[:, s4 * 128 : (s4 + 1) * 128], ident_b
                            )
                        nc.vector.tensor_copy(
                            out=ob[:, :, oc * 128 : (oc + 1) * 128], in_=pot_t
                        )
                        nc.gpsimd.dma_start(out=y_blk_ap(b, t8), in_=ob)
                    else:
                        eng = nc.sync if oc == 0 else nc.scalar
                        eng.dma_start_transpose(
                            out=ob[:, :, oc * 128 : (oc + 1) * 128], in_=osq
                        )
                        if oc == 1:
                            nc.gpsimd.dma_start(out=y_blk_ap(b, t8), in_=ob)

            def conv_chunk(c, oc, vt):
                mp = [None] * 4
                for p in PLANE_ORDER:
                    ps = pconv.tile([128, 512], f32, tag="pconv")
                    ps_r = r3(ps)
                    i = 0
                    for dx in [0, -1, 1]:  # dx=0 first: start=True covers all 512
                        for cc in range(2):
                            lhsT = up[:, cc, dx + 1, p, oc * 128 : (oc + 1) * 128]
                            vpl = r3(vt[:, p, cc])
                            if dx == 0:
                                rhs = vt[:, p, cc]
                                out_ap = ps
                            elif dx == -1:
                                rhs = vpl[:, :, 0:63]
                                out_ap = ps_r[:, :, 1:64]
                            else:
                                rhs = vpl[:, :, 1:64]
                                out_ap = ps_r[:, :, 0:63]
                            nc.tensor.matmul(
                                out_ap, lhsT, rhs, start=(i == 0), stop=(i == 5)
                            )
                            i += 1
                    msb = mpool.tile([128, 512], f16, tag="msb", name=f"m_{p}")
                    nc.scalar.activation(msb, ps, AF.Copy, scale=d_sb[:, oc : oc + 1])
                    mp[p] = msb
                evict_chunk(c, oc, mp)

            # ingest via PE transpose; V chunk c needs x blocks 0..2c+2
            vts = [None] * NCHUNK
            for t8 in range(3):
                transpose_block_pe(t8)
            vts[0] = build_v_chunk(0)
            conv_chunk(0, 0, vts[0])
            if b == 0 and BPC > 1:
                setup_sample(1)
            for t8 in range(3, 5):
                transpose_block_pe(t8)
            vts[1] = build_v_chunk(1)
            conv_chunk(0, 1, vts[0])
            conv_chunk(1, 0, vts[1])
            for t8 in range(5, 7):
                transpose_block_pe(t8)
            vts[2] = build_v_chunk(2)
            conv_chunk(1, 1, vts[1])
            conv_chunk(2, 0, vts[2])
            transpose_block_pe(7)
            vts[3] = build_v_chunk(3)
            conv_chunk(2, 1, vts[2])
            conv_chunk(3, 0, vts[3])
            conv_chunk(3, 1, vts[3])

    nc.compile()
    return nc


def _get_nc():
    if "nc" not in _CACHE:
        _CACHE["nc"] = _build_nc()
    return _CACHE["nc"]


def kernel(x, style, kernel, _trace=False):
    global LAST_EXEC_NS, LAST_MEAN_EXEC_NS
    from concourse.bass_utils import run_bass_kernel_spmd

    x = np.ascontiguousarray(x, dtype=np.float32)
    style = np.ascontiguousarray(style, dtype=np.float32)
    kern = np.ascontiguousarray(kernel, dtype=np.float32)

    nc = _get_nc()
    in_maps = [
        {
            "x": x[i * BPC : (i + 1) * BPC],
            "style": style[i * BPC : (i + 1) * BPC],
            "kernel": kern,
        }
        for i in range(NCORES)
    ]
    res = run_bass_kernel_spmd(nc, in_maps, core_ids=list(range(NCORES)), trace=_trace)
    LAST_EXEC_NS = res.exec_time_ns
    LAST_MEAN_EXEC_NS = res.mean_exec_time_ns
    return np.concatenate([res.results[i]["y"] for i in range(NCORES)], axis=0)


# revision 17
# speedup vs baseline: 1.2674x; 1.0233x over previous
"""Trainium2 Bass kernel for StyleGAN2-style modulated conv2d (ModConv2D).

Reference computation (per sample b):
    w      = kernel * (style[b] + 1)                 # modulate [3,3,Cin,Cout]
    w      = w / sqrt(sum(w^2, (kh,kw,Cin)) + 1e-8)  # demodulate per Cout
    y[b]   = conv2d_same(x[b], w)

Sharding: data-parallel over batch - 16 samples across 8 NeuronCores,
2 samples per core; the base kernel is replicated.

Algorithm: 1D Winograd F(2,3) along dy. Per sample the conv becomes
4 Winograd planes x 3 dx taps x 2 cin-chunks x 4 chunks x 2 cout-chunks
= 192 matmuls of N=512 (vs 288 for the direct 9-tap form), with cheap
transforms (plain adds/subs, DVE-friendly):
  U planes (lhsT, fp16): U0=w0, U1=(w0+w1+w2)/2, U2=(w0-w1+w2)/2, U3=w2
  V planes (rhs,  fp16): per row-pair rg (rows 2rg-1..2rg+2 of x):
      V0=d0-d2, V1=d1+d2, V2=d2-d1, V3=d1-d3
  M_p = sum_dx U_p[dx] @ V_p[.,c+dx]   (psum f32; dx=+-1 via the
      column-split trick: rhs [128,8,63], strided psum out)
  y[2rg+0] = (M0+M1+M2) * d ; y[2rg+1] = (M1-M2-M3) * d
      (ACT evicts M*d to fp16, DVE combines into the strided o_sb)

x ingest (fp32->fp16 SWDGE cast + PE transpose) and the output path
(xbar out-transpose, SWDGE fp32 store) follow the direct-conv kernel.
"""

import numpy as np

B, H, W, CIN, COUT, KH, KW = 16, 64, 64, 256, 256, 3, 3
NCORES = 8
BPC = B // NCORES  # samples per core
HWPIX = H * W  # 4096
PAD0 = 64  # one zero guard row before the image
XLEN = PAD0 + HWPIX + 256  # room for the row-strided V views
NCHUNK = 4  # chunks of 8 row-pairs (512 psum cols, 16 output rows) each

_CACHE = {}
LAST_EXEC_NS = None
LAST_MEAN_EXEC_NS = None


def _build_nc():
    from contextlib import ExitStack

    import concourse.bacc as bacc
    import concourse.bass as bass
    import concourse.mybir as mybir
    import concourse.tile as tile
    from concourse.masks import make_identity

    f32 = mybir.dt.float32
    f16 = mybir.dt.float16
    AF = mybir.ActivationFunctionType
    OP = mybir.AluOpType

    nc = bacc.Bacc("TRN2", target_bir_lowering=False, debug=False)

    x_d = nc.dram_tensor("x", [BPC, H, W, CIN], f32, kind="ExternalInput")
    s_d = nc.dram_tensor("style", [BPC, CIN], f32, kind="ExternalInput")
    k_d = nc.dram_tensor("kernel", [KH, KW, CIN, COUT], f32, kind="ExternalInput")
    y_d = nc.dram_tensor("y", [BPC, H, W, COUT], f32, kind="ExternalOutput")

    XB = H * W * CIN
    KKW = CIN * COUT

    def x_blk_ap(b, t8):
        # [128 pix, 2 cc, 4 sblk, 128 ci]: cc-major so xtmp[:, cc] is 2D-mergeable
        off = b * XB + t8 * 4 * 128 * CIN
        return bass.AP(
            x_d, off, [[CIN, 128], [128, 2], [128 * CIN, 4], [1, 128]]
        )

    def y_blk_ap(b, t8):
        off = b * XB + t8 * 4 * 128 * COUT
        return bass.AP(y_d, off, [[COUT, 128], [128 * COUT, 4], [1, COUT]])

    def k_tap_ap(cc, t):
        return bass.AP(k_d, t * KKW + cc * 128 * COUT, [[COUT, 128], [1, COUT]])

    # kernel tap DMA order: dy=0 row, then dy=2 (planes 0/3 are plain taps and
    # unblock the first conv planes), then dy=1
    KTAP_ORDER = [0, 1, 2, 6, 7, 8, 3, 4, 5]
    # conv plane order: view-planes first (their weights are ready earliest)
    PLANE_ORDER = [0, 3, 1, 2]

    def r3(a):
        return a.rearrange("p (r w) -> p r w", w=64)

    with tile.TileContext(nc) as tc, ExitStack() as ctx:
        singles = ctx.enter_context(tc.tile_pool(name="singles", bufs=1))
        tmp_pool = ctx.enter_context(tc.tile_pool(name="tmp", bufs=1))
        wmod_pool = ctx.enter_context(tc.tile_pool(name="wmod", bufs=2))
        upool = ctx.enter_context(tc.tile_pool(name="upool", bufs=2))
        dpool = ctx.enter_context(tc.tile_pool(name="dpool", bufs=2))
        srow_pool = ctx.enter_context(tc.tile_pool(name="srow", bufs=2))
        xpool = ctx.enter_context(tc.tile_pool(name="xpool", bufs=1))
        xtpool = ctx.enter_context(tc.tile_pool(name="xt", bufs=2 * 8))
        vpool = ctx.enter_context(tc.tile_pool(name="vpool", bufs=2))
        mpool = ctx.enter_context(tc.tile_pool(name="mpool", bufs=8))
        ytmp_pool = ctx.enter_context(tc.tile_pool(name="ytmp", bufs=1))
        ospool = ctx.enter_context(tc.tile_pool(name="osb", bufs=2))
        obpool = ctx.enter_context(tc.tile_pool(name="ob", bufs=8))
        pconv = ctx.enter_context(tc.tile_pool(name="pconv", bufs=5, space="PSUM"))
        pxt = ctx.enter_context(tc.tile_pool(name="pxt", bufs=2, space="PSUM"))
        psmall = ctx.enter_context(tc.tile_pool(name="psmall", bufs=1, space="PSUM"))

        # style rows + kernel tap loads, alternating HWDGE rings
        srows = []
        for b in range(BPC):
            srow = srow_pool.tile([1, CIN], f32, tag="srow")
            nc.scalar.dma_start(out=srow, in_=s_d.ap()[b : b + 1, :])
            srows.append(srow)
        kbase = singles.tile([128, 2, KH * KW, COUT], f32)
        for ti, t in enumerate(KTAP_ORDER):
            for cc in range(2):
                eng = nc.sync if (ti * 2 + cc) % 2 == 0 else nc.scalar
                eng.dma_start(out=kbase[:, cc, t], in_=k_tap_ap(cc, t))

        # x loads (fp32->fp16 SWDGE cast) issued upfront
        xts = [[None] * 8 for _ in range(BPC)]

        def load_xtmp(b, t8):
            xtmp = xtpool.tile([128, 2, 4, 128], f16, tag="xtmp", name=f"xtmp_{b}_{t8}")
            nc.gpsimd.dma_start(out=xtmp, in_=x_blk_ap(b, t8))
            xts[b][t8] = xtmp

        load_xtmp(0, 0)
        load_xtmp(0, 1)
        ident_b = singles.tile([128, 128], f16)
        make_identity(nc, ident_b)
        for b in range(BPC):
            for t8 in range(8):
                if xts[b][t8] is None:
                    load_xtmp(b, t8)

        ones1 = singles.tile([1, 1], f32)
        nc.vector.memset(ones1, 1.0)
        eps_sb = singles.tile([128, 1], f32)
        nc.vector.memset(eps_sb, 1e-8)

        # K2[cin, cout] = sum_t kernel^2 (for the demod factor); emitted
        # lazily so its DVE reduce doesn't outprioritize the first V build
        k2 = singles.tile([128, 2, COUT], f32)

        def compute_k2():
            for cc in range(2):
                k2tmp = tmp_pool.tile([128, KH * KW, COUT], f32)
                nc.vector.tensor_mul(k2tmp, kbase[:, cc], kbase[:, cc])
                nc.vector.reduce_sum(
                    out=k2[:, cc],
                    in_=k2tmp.rearrange("p t c -> p c t"),
                    axis=mybir.AxisListType.X,
                )

        ups, dsbs, s2cs = [None] * BPC, [None] * BPC, [None] * BPC

        def setup_mod(b):
            # modulation + U planes for sample b; planes 0/3 are views of wmod16
            srow1 = srow_pool.tile([1, CIN], f32, tag="srow1")
            nc.vector.tensor_scalar_add(srow1, srows[b], 1.0)

            smod = dpool.tile([128, 2], f32)
            s2c = dpool.tile([128, 2], f32)
            for cc in range(2):
                pcol = psmall.tile([128, 1], f32, tag="psmall")
                nc.tensor.matmul(
                    pcol, srow1[:, cc * 128 : (cc + 1) * 128], ones1, start=True, stop=True
                )
                nc.vector.tensor_copy(out=smod[:, cc : cc + 1], in_=pcol)
            nc.vector.tensor_mul(s2c, smod, smod)
            s2cs[b] = s2c

            wmod16 = wmod_pool.tile([128, 2, KH * KW, COUT], f16, tag="wmod16")
            up = upool.tile([128, 2, KW, 2, COUT], f16, tag="up")  # planes 1/2 only
            for t in KTAP_ORDER:
                for cc in range(2):
                    nc.scalar.activation(
                        wmod16[:, cc, t], kbase[:, cc, t], AF.Copy,
                        scale=smod[:, cc : cc + 1],
                    )
            # U1/U2 combos on DVE, whole dy-rows at a time ([128, 3dx, 256] fp16)
            for cc in range(2):
                w0 = wmod16[:, cc, 0:3]
                w1 = wmod16[:, cc, 3:6]
                w2 = wmod16[:, cc, 6:9]
                w1h = tmp_pool.tile([128, 3, COUT], f16, tag="u_w1h")
                nc.vector.tensor_scalar_mul(w1h, w1, 0.5)
                t02 = tmp_pool.tile([128, 3, COUT], f16, tag="u_t02")
                nc.vector.tensor_add(t02, w0, w2)
                nc.vector.scalar_tensor_tensor(
                    out=up[:, cc, :, 0, :], in0=t02, scalar=0.5, in1=w1h,
                    op0=OP.mult, op1=OP.add,
                )
                nc.vector.scalar_tensor_tensor(
                    out=up[:, cc, :, 1, :], in0=t02, scalar=0.5, in1=w1h,
                    op0=OP.mult, op1=OP.subtract,
                )
            ups[b] = (wmod16, up)

        def setup_demod(b):
            # demod d[cout] = rsqrt(sum_cc s2c^T @ k2 + 1e-8)
            s2c = s2cs[b]
            prow = psmall.tile([1, COUT], f32, tag="psmall")
            for cc in range(2):
                nc.tensor.matmul(
                    prow, s2c[:, cc : cc + 1], k2[:, cc], start=(cc == 0), stop=(cc == 1)
                )
            ssq_row = srow_pool.tile([1, COUT], f32, tag="ssq")
            nc.vector.tensor_copy(out=ssq_row, in_=prow)
            sqc = dpool.tile([128, 2], f32)
            for oc in range(2):
                pcol2 = psmall.tile([128, 1], f32, tag="psmall")
                nc.tensor.matmul(
                    pcol2, ssq_row[:, oc * 128 : (oc + 1) * 128], ones1, start=True, stop=True
                )
                nc.scalar.activation(sqc[:, oc : oc + 1], pcol2, AF.Sqrt, bias=eps_sb)
            d_sb = dpool.tile([128, 2], f32)
            nc.vector.reciprocal(d_sb, sqc)
            dsbs[b] = d_sb

        compute_k2()
        for _b in range(BPC):
            setup_mod(_b)
            setup_demod(_b)

        for b in range(BPC):
            wmod16, up = ups[b]
            d_sb = dsbs[b]
            # x channel-major flat: [128 cin, cc, XLEN] fp16; guard rows zero.
            # Single buffer: sample b+1's transposes depend on b's V reads.
            xflat = xpool.tile([128, 2, XLEN], f16, tag="xflat")
            if b == 0:
                nc.vector.memset(xflat[:, :, 0:PAD0], 0.0)
                nc.vector.memset(xflat[:, :, PAD0 + HWPIX : XLEN], 0.0)

            def transpose_block_pe(t8):
                xtmp = xts[b][t8]
                for cc in range(2):
                    pxt_t = pxt.tile([128, 4, 128], f16, tag="pxt")
                    for s4 in range(4):
                        nc.tensor.transpose(
                            pxt_t[:, s4, :], xtmp[:, cc, s4, :], ident_b
                        )
                    nc.vector.tensor_copy(
                        out=xflat[:, cc, PAD0 + 512 * t8 : PAD0 + 512 * (t8 + 1)],
                        in_=pxt_t,
                    )

            def build_v_chunk(c):
                # V planes for row-pairs 8c..8c+7 (output rows 16c..16c+15):
                # 4 plain tensor ops per cc over [128, 8, 64] row-strided views
                # of xflat (guard rows supply the SAME padding)
                vt = vpool.tile([128, 4, 2, 512], f16, tag="vt", name=f"vt_{b}_{c}")
                for cc in range(2):
                    xfc = xflat[:, cc]

                    def drow(k):
                        # rows 2rg + k - 1 for rg = 8c..8c+7 -> [128, 8(stride 128), 64]
                        base = PAD0 + (16 * c + k - 1) * 64
                        return xfc[:, base : base + 1024].rearrange(
                            "p (r f w) -> p r f w", f=2, w=64
                        )[:, :, 0, :]

                    def vplane(p):
                        return r3(vt[:, p, cc])

                    d0, d1, d2, d3 = (drow(k) for k in range(4))
                    nc.vector.tensor_sub(vplane(0), d0, d2)
                    nc.vector.tensor_add(vplane(1), d1, d2)
                    nc.vector.tensor_sub(vplane(2), d2, d1)
                    nc.vector.tensor_sub(vplane(3), d1, d3)
                return vt

            # output ob tiles, shared between the oc halves
            obs = {}
            for t8 in range(8):
                obs[t8] = obpool.tile([128, 4, COUT], f16, tag="ob", name=f"ob_{b}_{t8}")

            def evict_chunk(c, oc, mp):
                # mp: 4 sbuf fp16 tiles [128, 512] (already scaled by d)
                # y(2rg+0) = m0+m1+m2 ; y(2rg+1) = m1-m2-m3
                s_ = ytmp_pool.tile([128, 512], f16, tag="y_s")
                t_ = ytmp_pool.tile([128, 512], f16, tag="y_t")
                nc.vector.tensor_add(s_, mp[1], mp[2])
                nc.vector.tensor_sub(t_, mp[1], mp[2])
                o_sb = ospool.tile([128, 2 * 512], f16, tag="osb")
                o_v = o_sb.rearrange("p (r i w) -> p r i w", i=2, w=64)
                nc.vector.tensor_add(o_v[:, :, 0], r3(mp[0]), r3(s_))
                nc.vector.tensor_sub(o_v[:, :, 1], r3(t_), r3(mp[3]))
                # out-transpose + store, 512 px at a time
                last = b == BPC - 1 and c == NCHUNK - 1 and oc == 1
                for q in range(2):
                    t8 = c * 2 + q
                    osq = o_sb[:, q * 512 : (q + 1) * 512]
                    ob = obs[t8]
                    if last and q == 1:
                        # final tile: PE transpose (ingest psum pool idle now)
                        pot_t = pxt.tile([128, 4, 128], f16, tag="pxt")
                        for s4 in range(4):
                            nc.tensor.transpose(
                                pot_t[:, s4, :], osq[:, s4 * 128 : (s4 + 1) * 128], ident_b
                            )
                        nc.vector.tensor_copy(
                            out=ob[:, :, oc * 128 : (oc + 1) * 128], in_=pot_t
                        )
                        nc.gpsimd.dma_start(out=y_blk_ap(b, t8), in_=ob)
                    else:
                        eng = nc.sync if oc == 0 else nc.scalar
                        eng.dma_start_transpose(
                            out=ob[:, :, oc * 128 : (oc + 1) * 128], in_=osq
                        )
                        if oc == 1:
                            nc.gpsimd.dma_start(out=y_blk_ap(b, t8), in_=ob)

            def conv_chunk(c, oc, vt):
                mp = [None] * 4
                for p in PLANE_ORDER:
                    ps = pconv.tile([128, 512], f32, tag="pconv")
                    ps_r = r3(ps)
                    i = 0
                    for dx in [0, -1, 1]:  # dx=0 first: start=True covers all 512
                        for cc in range(2):
                            if p == 0:
                                lhsT = wmod16[:, cc, dx + 1, oc * 128 : (oc + 1) * 128]
                            elif p == 3:
                                lhsT = wmod16[:, cc, 6 + dx + 1, oc * 128 : (oc + 1) * 128]
                            else:
                                lhsT = up[:, cc, dx + 1, p - 1, oc * 128 : (oc + 1) * 128]
                            vpl = r3(vt[:, p, cc])
                            if dx == 0:
                                rhs = vt[:, p, cc]
                                out_ap = ps
                            elif dx == -1:
                                rhs = vpl[:, :, 0:63]
                                out_ap = ps_r[:, :, 1:64]
                            else:
                                rhs = vpl[:, :, 1:64]
                                out_ap = ps_r[:, :, 0:63]
                            nc.tensor.matmul(
                                out_ap, lhsT, rhs, start=(i == 0), stop=(i == 5)
                            )
                            i += 1
                    msb = mpool.tile([128, 512], f16, tag="msb", name=f"m_{p}")
                    nc.scalar.activation(msb, ps, AF.Copy, scale=d_sb[:, oc : oc + 1])
                    mp[p] = msb
                evict_chunk(c, oc, mp)

            # ingest via PE transpose, then chunk-paced V builds
            for t8 in range(8):
                transpose_block_pe(t8)
            vts = [None] * NCHUNK
            vts[0] = build_v_chunk(0)
            conv_chunk(0, 0, vts[0])
            vts[1] = build_v_chunk(1)
            conv_chunk(0, 1, vts[0])
            conv_chunk(1, 0, vts[1])
            vts[2] = build_v_chunk(2)
            conv_chunk(1, 1, vts[1])
            conv_chunk(2, 0, vts[2])
            vts[3] = build_v_chunk(3)
            conv_chunk(2, 1, vts[2])
            conv_chunk(3, 0, vts[3])
            conv_chunk(3, 1, vts[3])

    nc.compile()
    return nc


def _get_nc():
    if "nc" not in _CACHE:
        _CACHE["nc"] = _build_nc()
    return _CACHE["nc"]


def kernel(x, style, kernel, _trace=False):
    global LAST_EXEC_NS, LAST_MEAN_EXEC_NS
    from concourse.bass_utils import run_bass_kernel_spmd

    x = np.ascontiguousarray(x, dtype=np.float32)
    style = np.ascontiguousarray(style, dtype=np.float32)
    kern = np.ascontiguousarray(kernel, dtype=np.float32)

    nc = _get_nc()
    in_maps = [
        {
            "x": x[i * BPC : (i + 1) * BPC],
            "style": style[i * BPC : (i + 1) * BPC],
            "kernel": kern,
        }
        for i in range(NCORES)
    ]
    res = run_bass_kernel_spmd(nc, in_maps, core_ids=list(range(NCORES)), trace=_trace)
    LAST_EXEC_NS = res.exec_time_ns
    LAST_MEAN_EXEC_NS = res.mean_exec_time_ns
    return np.concatenate([res.results[i]["y"] for i in range(NCORES)], axis=0)


# revision 18
# speedup vs baseline: 1.3177x; 1.0397x over previous
"""Trainium2 Bass kernel for StyleGAN2-style modulated conv2d (ModConv2D).

Reference computation (per sample b):
    w      = kernel * (style[b] + 1)                 # modulate [3,3,Cin,Cout]
    w      = w / sqrt(sum(w^2, (kh,kw,Cin)) + 1e-8)  # demodulate per Cout
    y[b]   = conv2d_same(x[b], w)

Sharding: data-parallel over batch - 16 samples across 8 NeuronCores,
2 samples per core; the base kernel is replicated.

Algorithm: 1D Winograd F(2,3) along dy. Per sample the conv becomes
4 Winograd planes x 3 dx taps x 2 cin-chunks x 4 chunks x 2 cout-chunks
= 192 matmuls of N=512 (vs 288 for the direct 9-tap form), with cheap
transforms (plain adds/subs, DVE-friendly):
  U planes (lhsT, fp16): U0=w0, U1=(w0+w1+w2)/2, U2=(w0-w1+w2)/2, U3=w2
  V planes (rhs,  fp16): per row-pair rg (rows 2rg-1..2rg+2 of x):
      V0=d0-d2, V1=d1+d2, V2=d2-d1, V3=d1-d3
  M_p = sum_dx U_p[dx] @ V_p[.,c+dx]   (psum f32; dx=+-1 via the
      column-split trick: rhs [128,8,63], strided psum out)
  y[2rg+0] = (M0+M1+M2) * d ; y[2rg+1] = (M1-M2-M3) * d
      (ACT evicts M*d to fp16, DVE combines into the strided o_sb)

x ingest (fp32->fp16 SWDGE cast + PE transpose) and the output path
(xbar out-transpose, SWDGE fp32 store) follow the direct-conv kernel.
"""

import numpy as np

B, H, W, CIN, COUT, KH, KW = 16, 64, 64, 256, 256, 3, 3
NCORES = 8
BPC = B // NCORES  # samples per core
HWPIX = H * W  # 4096
PAD0 = 64  # one zero guard row before the image
XLEN = PAD0 + HWPIX + 256  # room for the row-strided V views
NCHUNK = 4  # chunks of 8 row-pairs (512 psum cols, 16 output rows) each

_CACHE = {}
LAST_EXEC_NS = None
LAST_MEAN_EXEC_NS = None


def _build_nc():
    from contextlib import ExitStack

    import concourse.bacc as bacc
    import concourse.bass as bass
    import concourse.mybir as mybir
    import concourse.tile as tile
    from concourse.masks import make_identity

    f32 = mybir.dt.float32
    f16 = mybir.dt.float16
    AF = mybir.ActivationFunctionType
    OP = mybir.AluOpType

    nc = bacc.Bacc("TRN2", target_bir_lowering=False, debug=False)

    x_d = nc.dram_tensor("x", [BPC, H, W, CIN], f32, kind="ExternalInput")
    s_d = nc.dram_tensor("style", [BPC, CIN], f32, kind="ExternalInput")
    k_d = nc.dram_tensor("kernel", [KH, KW, CIN, COUT], f32, kind="ExternalInput")
    y_d = nc.dram_tensor("y", [BPC, H, W, COUT], f32, kind="ExternalOutput")

    XB = H * W * CIN
    KKW = CIN * COUT

    def x_blk_ap(b, t8):
        # [128 pix, 2 cc, 4 sblk, 128 ci]: cc-major so xtmp[:, cc] is 2D-mergeable
        off = b * XB + t8 * 4 * 128 * CIN
        return bass.AP(
            x_d, off, [[CIN, 128], [128, 2], [128 * CIN, 4], [1, 128]]
        )

    def y_blk_ap(b, t8):
        off = b * XB + t8 * 4 * 128 * COUT
        return bass.AP(y_d, off, [[COUT, 128], [128 * COUT, 4], [1, COUT]])

    def k_tap_ap(cc, t):
        return bass.AP(k_d, t * KKW + cc * 128 * COUT, [[COUT, 128], [1, COUT]])

    # kernel tap DMA order: dy=0 row, then dy=2 (planes 0/3 are plain taps and
    # unblock the first conv planes), then dy=1
    KTAP_ORDER = [0, 1, 2, 6, 7, 8, 3, 4, 5]
    # conv plane order: view-planes first (their weights are ready earliest)
    PLANE_ORDER = [0, 3, 1, 2]

    def r3(a):
        return a.rearrange("p (r w) -> p r w", w=64)

    with tile.TileContext(nc) as tc, ExitStack() as ctx:
        singles = ctx.enter_context(tc.tile_pool(name="singles", bufs=1))
        tmp_pool = ctx.enter_context(tc.tile_pool(name="tmp", bufs=1))
        wmod_pool = ctx.enter_context(tc.tile_pool(name="wmod", bufs=2))
        upool = ctx.enter_context(tc.tile_pool(name="upool", bufs=2))
        dpool = ctx.enter_context(tc.tile_pool(name="dpool", bufs=2))
        srow_pool = ctx.enter_context(tc.tile_pool(name="srow", bufs=2))
        xpool = ctx.enter_context(tc.tile_pool(name="xpool", bufs=1))
        xtpool = ctx.enter_context(tc.tile_pool(name="xt", bufs=2 * 8))
        vpool = ctx.enter_context(tc.tile_pool(name="vpool", bufs=2))
        mpool = ctx.enter_context(tc.tile_pool(name="mpool", bufs=8))
        ytmp_pool = ctx.enter_context(tc.tile_pool(name="ytmp", bufs=1))
        ospool = ctx.enter_context(tc.tile_pool(name="osb", bufs=2))
        obpool = ctx.enter_context(tc.tile_pool(name="ob", bufs=8))
        pconv = ctx.enter_context(tc.tile_pool(name="pconv", bufs=5, space="PSUM"))
        pxt = ctx.enter_context(tc.tile_pool(name="pxt", bufs=2, space="PSUM"))
        psmall = ctx.enter_context(tc.tile_pool(name="psmall", bufs=1, space="PSUM"))

        # style rows + kernel tap loads, alternating HWDGE rings
        srows = []
        for b in range(BPC):
            srow = srow_pool.tile([1, CIN], f32, tag="srow")
            nc.scalar.dma_start(out=srow, in_=s_d.ap()[b : b + 1, :])
            srows.append(srow)
        kbase = singles.tile([128, 2, KH * KW, COUT], f32)
        for ti, t in enumerate(KTAP_ORDER):
            for cc in range(2):
                eng = nc.sync if (ti * 2 + cc) % 2 == 0 else nc.scalar
                eng.dma_start(out=kbase[:, cc, t], in_=k_tap_ap(cc, t))

        # x loads (fp32->fp16 SWDGE cast) issued upfront
        xts = [[None] * 8 for _ in range(BPC)]

        def load_xtmp(b, t8):
            xtmp = xtpool.tile([128, 2, 4, 128], f16, tag="xtmp", name=f"xtmp_{b}_{t8}")
            nc.gpsimd.dma_start(out=xtmp, in_=x_blk_ap(b, t8))
            xts[b][t8] = xtmp

        load_xtmp(0, 0)
        load_xtmp(0, 1)
        ident_b = singles.tile([128, 128], f16)
        make_identity(nc, ident_b)
        for b in range(BPC):
            for t8 in range(8):
                if xts[b][t8] is None:
                    load_xtmp(b, t8)

        ones1 = singles.tile([1, 1], f32)
        nc.vector.memset(ones1, 1.0)
        eps_sb = singles.tile([128, 1], f32)
        nc.vector.memset(eps_sb, 1e-8)

        # K2[cin, cout] = sum_t kernel^2 (for the demod factor); emitted
        # lazily so its DVE reduce doesn't outprioritize the first V build
        k2 = singles.tile([128, 2, COUT], f32)

        def compute_k2():
            for cc in range(2):
                k2tmp = tmp_pool.tile([128, KH * KW, COUT], f32)
                nc.vector.tensor_mul(k2tmp, kbase[:, cc], kbase[:, cc])
                nc.vector.reduce_sum(
                    out=k2[:, cc],
                    in_=k2tmp.rearrange("p t c -> p c t"),
                    axis=mybir.AxisListType.X,
                )

        ups, dsbs, s2cs = [None] * BPC, [None] * BPC, [None] * BPC

        def setup_mod(b):
            # modulation + U planes for sample b; planes 0/3 are views of wmod16
            srow1 = srow_pool.tile([1, CIN], f32, tag="srow1")
            nc.vector.tensor_scalar_add(srow1, srows[b], 1.0)

            smod = dpool.tile([128, 2], f32)
            s2c = dpool.tile([128, 2], f32)
            for cc in range(2):
                pcol = psmall.tile([128, 1], f32, tag="psmall")
                nc.tensor.matmul(
                    pcol, srow1[:, cc * 128 : (cc + 1) * 128], ones1, start=True, stop=True
                )
                nc.vector.tensor_copy(out=smod[:, cc : cc + 1], in_=pcol)
            nc.vector.tensor_mul(s2c, smod, smod)
            s2cs[b] = s2c

            wmod16 = wmod_pool.tile([128, 2, KH * KW, COUT], f16, tag="wmod16")
            up = upool.tile([128, 2, KW, 2, COUT], f16, tag="up")  # planes 1/2 only
            for t in KTAP_ORDER:
                for cc in range(2):
                    nc.scalar.activation(
                        wmod16[:, cc, t], kbase[:, cc, t], AF.Copy,
                        scale=smod[:, cc : cc + 1],
                    )
            # U1/U2 combos on DVE, whole dy-rows at a time ([128, 3dx, 256] fp16)
            for cc in range(2):
                w0 = wmod16[:, cc, 0:3]
                w1 = wmod16[:, cc, 3:6]
                w2 = wmod16[:, cc, 6:9]
                w1h = tmp_pool.tile([128, 3, COUT], f16, tag="u_w1h")
                nc.vector.tensor_scalar_mul(w1h, w1, 0.5)
                t02 = tmp_pool.tile([128, 3, COUT], f16, tag="u_t02")
                nc.vector.tensor_add(t02, w0, w2)
                nc.vector.scalar_tensor_tensor(
                    out=up[:, cc, :, 0, :], in0=t02, scalar=0.5, in1=w1h,
                    op0=OP.mult, op1=OP.add,
                )
                nc.vector.scalar_tensor_tensor(
                    out=up[:, cc, :, 1, :], in0=t02, scalar=0.5, in1=w1h,
                    op0=OP.mult, op1=OP.subtract,
                )
            ups[b] = (wmod16, up)

        def setup_demod(b):
            # demod d[cout] = rsqrt(sum_cc s2c^T @ k2 + 1e-8)
            s2c = s2cs[b]
            prow = psmall.tile([1, COUT], f32, tag="psmall")
            for cc in range(2):
                nc.tensor.matmul(
                    prow, s2c[:, cc : cc + 1], k2[:, cc], start=(cc == 0), stop=(cc == 1)
                )
            ssq_row = srow_pool.tile([1, COUT], f32, tag="ssq")
            nc.vector.tensor_copy(out=ssq_row, in_=prow)
            sqc = dpool.tile([128, 2], f32)
            for oc in range(2):
                pcol2 = psmall.tile([128, 1], f32, tag="psmall")
                nc.tensor.matmul(
                    pcol2, ssq_row[:, oc * 128 : (oc + 1) * 128], ones1, start=True, stop=True
                )
                nc.scalar.activation(sqc[:, oc : oc + 1], pcol2, AF.Sqrt, bias=eps_sb)
            d_sb = dpool.tile([128, 2], f32)
            nc.vector.reciprocal(d_sb, sqc)
            dsbs[b] = d_sb

        compute_k2()
        for _b in range(BPC):
            setup_mod(_b)
            setup_demod(_b)

        for b in range(BPC):
            wmod16, up = ups[b]
            d_sb = dsbs[b]
            # x channel-major flat: [128 cin, cc, XLEN] fp16; guard rows zero.
            # Single buffer: sample b+1's transposes depend on b's V reads.
            xflat = xpool.tile([128, 2, XLEN], f16, tag="xflat")
            if b == 0:
                nc.vector.memset(xflat[:, :, 0:PAD0], 0.0)
                nc.vector.memset(xflat[:, :, PAD0 + HWPIX : XLEN], 0.0)

            def transpose_block_pe(t8):
                xtmp = xts[b][t8]
                for cc in range(2):
                    pxt_t = pxt.tile([128, 4, 128], f16, tag="pxt")
                    for s4 in range(4):
                        nc.tensor.transpose(
                            pxt_t[:, s4, :], xtmp[:, cc, s4, :], ident_b
                        )
                    nc.vector.tensor_copy(
                        out=xflat[:, cc, PAD0 + 512 * t8 : PAD0 + 512 * (t8 + 1)],
                        in_=pxt_t,
                    )

            def build_v_chunk(c):
                # V planes for row-pairs 8c..8c+7 (output rows 16c..16c+15):
                # 4 plain tensor ops per cc over [128, 8, 64] row-strided views
                # of xflat (guard rows supply the SAME padding)
                vt = vpool.tile([128, 4, 2, 512], f16, tag="vt", name=f"vt_{b}_{c}")
                for cc in range(2):
                    xfc = xflat[:, cc]

                    def drow(k):
                        # rows 2rg + k - 1 for rg = 8c..8c+7 -> [128, 8(stride 128), 64]
                        base = PAD0 + (16 * c + k - 1) * 64
                        return xfc[:, base : base + 1024].rearrange(
                            "p (r f w) -> p r f w", f=2, w=64
                        )[:, :, 0, :]

                    def vplane(p):
                        return r3(vt[:, p, cc])

                    d0, d1, d2, d3 = (drow(k) for k in range(4))
                    nc.vector.tensor_sub(vplane(0), d0, d2)
                    nc.vector.tensor_add(vplane(1), d1, d2)
                    nc.vector.tensor_sub(vplane(2), d2, d1)
                    nc.vector.tensor_sub(vplane(3), d1, d3)
                return vt

            # output ob tiles, shared between the oc halves
            obs = {}
            for t8 in range(8):
                obs[t8] = obpool.tile([128, 4, COUT], f16, tag="ob", name=f"ob_{b}_{t8}")

            def evict_chunk(c, oc, mp):
                # mp: 4 sbuf fp16 tiles [128, 512] (already scaled by d)
                # y(2rg+0) = m0+m1+m2 ; y(2rg+1) = m1-m2-m3
                s_ = ytmp_pool.tile([128, 512], f16, tag="y_s")
                t_ = ytmp_pool.tile([128, 512], f16, tag="y_t")
                nc.vector.tensor_add(s_, mp[1], mp[2])
                nc.vector.tensor_sub(t_, mp[1], mp[2])
                o_sb = ospool.tile([128, 2 * 512], f16, tag="osb")
                o_v = o_sb.rearrange("p (r i w) -> p r i w", i=2, w=64)
                nc.vector.tensor_add(o_v[:, :, 0], r3(mp[0]), r3(s_))
                nc.vector.tensor_sub(o_v[:, :, 1], r3(t_), r3(mp[3]))
                # out-transpose + store, 512 px at a time
                last = b == BPC - 1 and c == NCHUNK - 1 and oc == 1
                for q in range(2):
                    t8 = c * 2 + q
                    osq = o_sb[:, q * 512 : (q + 1) * 512]
                    ob = obs[t8]
                    if last and q == 1:
                        # final tile: PE transpose (ingest psum pool idle now)
                        pot_t = pxt.tile([128, 4, 128], f16, tag="pxt")
                        for s4 in range(4):
                            nc.tensor.transpose(
                                pot_t[:, s4, :], osq[:, s4 * 128 : (s4 + 1) * 128], ident_b
                            )
                        nc.vector.tensor_copy(
                            out=ob[:, :, oc * 128 : (oc + 1) * 128], in_=pot_t
                        )
                        nc.gpsimd.dma_start(out=y_blk_ap(b, t8), in_=ob)
                    else:
                        eng = nc.sync if oc == 0 else nc.scalar
                        eng.dma_start_transpose(
                            out=ob[:, :, oc * 128 : (oc + 1) * 128], in_=osq
                        )
                        if oc == 1:
                            nc.gpsimd.dma_start(out=y_blk_ap(b, t8), in_=ob)

            def conv_chunk(c, oc, vt):
                mp = [None] * 4
                for p in PLANE_ORDER:
                    ps = pconv.tile([128, 512], f32, tag="pconv")
                    ps_r = r3(ps)
                    i = 0
                    for dx in [0, -1, 1]:  # dx=0 first: start=True covers all 512
                        for cc in range(2):
                            if p == 0:
                                lhsT = wmod16[:, cc, dx + 1, oc * 128 : (oc + 1) * 128]
                            elif p == 3:
                                lhsT = wmod16[:, cc, 6 + dx + 1, oc * 128 : (oc + 1) * 128]
                            else:
                                lhsT = up[:, cc, dx + 1, p - 1, oc * 128 : (oc + 1) * 128]
                            vpl = r3(vt[:, p, cc])
                            if dx == 0:
                                rhs = vt[:, p, cc]
                                out_ap = ps
                            elif dx == -1:
                                rhs = vpl[:, :, 0:63]
                                out_ap = ps_r[:, :, 1:64]
                            else:
                                rhs = vpl[:, :, 1:64]
                                out_ap = ps_r[:, :, 0:63]
                            nc.tensor.matmul(
                                out_ap, lhsT, rhs, start=(i == 0), stop=(i == 5)
                            )
                            i += 1
                    msb = mpool.tile([128, 512], f16, tag="msb", name=f"m_{p}")
                    nc.scalar.activation(msb, ps, AF.Copy, scale=d_sb[:, oc : oc + 1])
                    mp[p] = msb
                evict_chunk(c, oc, mp)

            def conv_chunk_split(c, oc, vt):
                # final chunk-oc: two half-width column groups so the first
                # half's evict/transpose/store pipeline overlaps the second
                # half's matmuls (shorter serial tail)
                for h in range(2):
                    mp = [None] * 4
                    for p in PLANE_ORDER:
                        ps = pconv.tile([128, 256], f32, tag="pconv")
                        ps_r = ps.rearrange("p (r w) -> p r w", w=64)
                        i = 0
                        for dx in [0, -1, 1]:
                            for cc in range(2):
                                if p == 0:
                                    lhsT = wmod16[:, cc, dx + 1, oc * 128 : (oc + 1) * 128]
                                elif p == 3:
                                    lhsT = wmod16[:, cc, 6 + dx + 1, oc * 128 : (oc + 1) * 128]
                                else:
                                    lhsT = up[:, cc, dx + 1, p - 1, oc * 128 : (oc + 1) * 128]
                                vpl = r3(vt[:, p, cc])[:, 4 * h : 4 * h + 4, :]
                                if dx == 0:
                                    rhs = vt[:, p, cc][:, 256 * h : 256 * (h + 1)]
                                    out_ap = ps
                                elif dx == -1:
                                    rhs = vpl[:, :, 0:63]
                                    out_ap = ps_r[:, :, 1:64]
                                else:
                                    rhs = vpl[:, :, 1:64]
                                    out_ap = ps_r[:, :, 0:63]
                                nc.tensor.matmul(
                                    out_ap, lhsT, rhs, start=(i == 0), stop=(i == 5)
                                )
                                i += 1
                        msb = mpool.tile([128, 256], f16, tag="msb2", name=f"m2_{p}")
                        nc.scalar.activation(
                            msb, ps, AF.Copy, scale=d_sb[:, oc : oc + 1]
                        )
                        mp[p] = msb
                    s_ = ytmp_pool.tile([128, 256], f16, tag="y_s2")
                    t_ = ytmp_pool.tile([128, 256], f16, tag="y_t2")
                    nc.vector.tensor_add(s_, mp[1], mp[2])
                    nc.vector.tensor_sub(t_, mp[1], mp[2])
                    o_sb = ospool.tile([128, 512], f16, tag="osb2")
                    o_v = o_sb.rearrange("p (r i w) -> p r i w", i=2, w=64)
                    nc.vector.tensor_add(o_v[:, :, 0], r3(mp[0]), r3(s_))
                    nc.vector.tensor_sub(o_v[:, :, 1], r3(t_), r3(mp[3]))
                    t8 = c * 2 + h
                    ob = obs[t8]
                    if h == 1:
                        pot_t = pxt.tile([128, 4, 128], f16, tag="pxt")
                        for s4 in range(4):
                            nc.tensor.transpose(
                                pot_t[:, s4, :], o_sb[:, s4 * 128 : (s4 + 1) * 128], ident_b
                            )
                        nc.vector.tensor_copy(
                            out=ob[:, :, oc * 128 : (oc + 1) * 128], in_=pot_t
                        )
                    else:
                        nc.scalar.dma_start_transpose(
                            out=ob[:, :, oc * 128 : (oc + 1) * 128], in_=o_sb
                        )
                    nc.gpsimd.dma_start(out=y_blk_ap(b, t8), in_=ob)

            # ingest via PE transpose, then chunk-paced V builds
            for t8 in range(8):
                transpose_block_pe(t8)
            vts = [None] * NCHUNK
            vts[0] = build_v_chunk(0)
            conv_chunk(0, 0, vts[0])
            vts[1] = build_v_chunk(1)
            conv_chunk(0, 1, vts[0])
            conv_chunk(1, 0, vts[1])
            vts[2] = build_v_chunk(2)
            conv_chunk(1, 1, vts[1])
            conv_chunk(2, 0, vts[2])
            vts[3] = build_v_chunk(3)
            conv_chunk(2, 1, vts[2])
            conv_chunk(3, 0, vts[3])
            if b == BPC - 1:
                conv_chunk_split(3, 1, vts[3])
            else:
                conv_chunk(3, 1, vts[3])

    nc.compile()
    return nc


def _get_nc():
    if "nc" not in _CACHE:
        _CACHE["nc"] = _build_nc()
    return _CACHE["nc"]


def kernel(x, style, kernel, _trace=False):
    global LAST_EXEC_NS, LAST_MEAN_EXEC_NS
    from concourse.bass_utils import run_bass_kernel_spmd

    x = np.ascontiguousarray(x, dtype=np.float32)
    style = np.ascontiguousarray(style, dtype=np.float32)
    kern = np.ascontiguousarray(kernel, dtype=np.float32)

    nc = _get_nc()
    in_maps = [
        {
            "x": x[i * BPC : (i + 1) * BPC],
            "style": style[i * BPC : (i + 1) * BPC],
            "kernel": kern,
        }
        for i in range(NCORES)
    ]
    res = run_bass_kernel_spmd(nc, in_maps, core_ids=list(range(NCORES)), trace=_trace)
    LAST_EXEC_NS = res.exec_time_ns
    LAST_MEAN_EXEC_NS = res.mean_exec_time_ns
    return np.concatenate([res.results[i]["y"] for i in range(NCORES)], axis=0)
